# revision 11
# baseline (speedup 1.0000x reference)
"""LTC/NCP RNN (BasicRNNClassifier) Trainium2 Bass kernel.

Strategy: pure data parallel over batch (256 -> 8 cores x 32).
Per core, the sequential T=4096 recurrence runs with:
  - synapse pairs (i,j) laid out on 121 SBUF partitions
  - PE matmuls for partition-broadcast of v (sigma folded into the
    broadcast matrix) and for the masked/weighted reductions over i
    (w*mask*(erev|1) folded into a constant [121,22] matrix)
  - ACT sigmoid with per-partition bias (-mu*sigma)
  - DVE for the semi-implicit Euler update (mul/add/reciprocal/mul)
  - sensory synapses are v-independent: batched per 16-step chunk
Host side: input affine + transposes, final gather at seq_lengths-1,
output affine + Dense(1).
"""

import numpy as np

U = 11
S = 15
F = 16
MOTOR = 1
UNFOLDS = 6
EPS = 1e-8
B, T = 256, 4096
NCORES = 8
BC = B // NCORES          # 32 batch per core
CHUNK = 16                # timesteps per loop iteration
W = CHUNK * BC            # 512 columns per chunk
NCH = T // CHUNK          # 256 chunks


# packed constant block: name -> (rows, col_offset, cols)
_sizes = [("sigB", U, U * U), ("gw", U * U, 43), ("i43", 43, 43),
          ("sigBsA", S, 88), ("sigBsB", S, 77), ("gwsA", 88, 43),
          ("gwsB", 77, 43), ("aug", 1, 43), ("cm6", 1, U),
          ("negmusig", U * U, 1), ("nmsA", 88, 1), ("nmsB", 77, 1)]
CB_LAYOUT = {}
_off = 0
for _n, _r, _c in _sizes:
    CB_LAYOUT[_n] = (_r, _off, _c)
    _off += _c
CB_COLS = _off

_cache = {}


def _build(t_steps, chunk):
    import concourse.bass as bass
    import concourse.tile as tile
    import concourse.mybir as mybir
    from concourse import bacc
    from contextlib import ExitStack

    import concourse.tile_sem_assignment as _tsa
    _tsa.NUM_HWDGE_SEMS = 1   # keep the loop back-edge barrier under the
                              # per-instruction sync-wait limit

    f32 = mybir.dt.float32
    nch = t_steps // chunk
    w = chunk * BC

    nc = bacc.Bacc("TRN2", target_bir_lowering=False, debug=False)

    xs_d = nc.dram_tensor("xs", [33, t_steps * BC], f32, kind="ExternalInput").ap()
    ys_d = nc.dram_tensor("ys", [1, t_steps * BC], f32, kind="ExternalOutput").ap()

    cb_d = nc.dram_tensor("cb", [128, CB_COLS], f32, kind="ExternalInput").ap()

    with ExitStack() as ctx:
        tc = ctx.enter_context(tile.TileContext(nc))

        cpool = ctx.enter_context(tc.tile_pool(name="consts", bufs=1))
        vpool = ctx.enter_context(tc.tile_pool(name="vstate", bufs=1))
        xpool = ctx.enter_context(tc.tile_pool(name="xin", bufs=2))
        spool = ctx.enter_context(tc.tile_pool(name="sens", bufs=2))
        ypool = ctx.enter_context(tc.tile_pool(name="yout", bufs=2))
        apool = ctx.enter_context(tc.tile_pool(name="acts", bufs=3))
        tpool = ctx.enter_context(tc.tile_pool(name="tmps", bufs=3))
        pp_s = ctx.enter_context(tc.tile_pool(name="ps_sens", bufs=1, space="PSUM"))
        pp_u = ctx.enter_context(tc.tile_pool(name="ps_unf", bufs=2, space="PSUM"))

        cb = cpool.tile([128, CB_COLS], f32, tag="cb")
        nc.sync.dma_start(cb[:], cb_d[:])
        c = {k: cb[0:r, o:o + n] for k, (r, o, n) in CB_LAYOUT.items()}

        ones = cpool.tile([1, w], f32, tag="ones")
        nc.vector.memset(ones[:], 1.0)
        va = vpool.tile([U, BC], f32, tag="va")
        vb = vpool.tile([U, BC], f32, tag="vb")
        nc.vector.memset(va[:], 0.0)

        sig = mybir.ActivationFunctionType.Sigmoid

        with tc.For_i(0, nch, 1,
                      hint_engines=(mybir.EngineType.PE, mybir.EngineType.DVE)) as ci:
            x_sb = xpool.tile([33, w], f32, tag="x")
            nc.sync.dma_start(x_sb[:], xs_d[:, bass.ts(ci, w)])

            # sensory synapses, batched over the whole chunk
            pA = pp_s.tile([88, w], f32, tag="pA")
            nc.tensor.matmul(pA[:], c["sigBsA"][:], x_sb[0:S, :], start=True, stop=True)
            aA = spool.tile([88, w], f32, tag="aA")
            nc.scalar.activation(aA[:], pA[:], sig, bias=c["nmsA"][:])
            pB = pp_s.tile([77, w], f32, tag="pB")
            nc.tensor.matmul(pB[:], c["sigBsB"][:], x_sb[0:S, :], start=True, stop=True)
            aB = spool.tile([77, w], f32, tag="aB")
            nc.scalar.activation(aB[:], pB[:], sig, bias=c["nmsB"][:])

            p_nd1 = pp_s.tile([43, w], f32, tag="pnd1")
            nc.tensor.matmul(p_nd1[:], c["gwsA"][:], aA[:], start=True, stop=False)
            nc.tensor.matmul(p_nd1[:], c["gwsB"][:], aB[:], start=False, stop=False)
            nc.tensor.matmul(p_nd1[:], c["aug"][:], ones[:], start=False, stop=True)

            # cm_t = UNFOLDS * cm / elapsed  (elapsed is input row 15)
            rec = tpool.tile([1, w], f32, tag="rec")
            nc.vector.reciprocal(rec[:], x_sb[32:33, :])
            p_cm = pp_s.tile([U, w], f32, tag="pcm")
            nc.tensor.matmul(p_cm[:], c["cm6"][:], rec[:], start=True, stop=True)
            cmt = spool.tile([U, w], f32, tag="cmt")
            nc.vector.tensor_copy(cmt[:], p_cm[:])

            nd1 = spool.tile([43, w], f32, tag="nd1")
            nc.vector.tensor_copy(nd1[:], p_nd1[:])
            nc.vector.tensor_add(nd1[32:43, :], p_nd1[32:43, :], cmt[:])

            ys_sb = ypool.tile([1, w], f32, tag="ys")

            vcur = va
            for s in range(chunk):
                col = slice(s * BC, (s + 1) * BC)
                for k in range(UNFOLDS):
                    p_nd = pp_u.tile([43, BC], f32, tag="pnd")
                    nc.tensor.matmul(p_nd[:], c["i43"][:], nd1[:, col],
                                     start=True, stop=False)
                    p_vr = pp_u.tile([U * U, BC], f32, tag="pvr")
                    nc.tensor.matmul(p_vr[:], c["sigB"][:], vcur[:],
                                     start=True, stop=True)
                    act = apool.tile([U * U, BC], f32, tag="act")
                    nc.scalar.activation(act[:], p_vr[:], sig, bias=c["negmusig"][:])
                    nc.tensor.matmul(p_nd[:], c["gw"][:], act[:],
                                     start=False, stop=True)

                    t1 = tpool.tile([U, BC], f32, tag="t1")
                    nc.vector.tensor_mul(t1[:], cmt[:, col], vcur[:])
                    numer = tpool.tile([U, BC], f32, tag="numer")
                    nc.vector.tensor_add(numer[:], t1[:], p_nd[0:U, :])
                    rcp = tpool.tile([U, BC], f32, tag="rcp")
                    nc.vector.reciprocal(rcp[:], p_nd[32:43, :])
                    vnext = vb if k % 2 == 0 else va
                    nc.vector.tensor_mul(vnext[:], numer[:], rcp[:])
                    vcur = vnext
                nc.scalar.copy(ys_sb[0:1, col], vcur[0:1, :])

            nc.sync.dma_start(ys_d[:, bass.ts(ci, w)], ys_sb[:])

    nc.compile()
    return nc


def _prep_consts(p):
    """Build the constant matrices from the parameter dict (numpy f32)."""
    iU = np.arange(U)
    sigB = np.zeros((U, U * U), np.float32)
    sigB[iU[:, None], iU[:, None] * U + iU[None, :]] = p["sigma"]
    negmusig = (-(p["mu"] * p["sigma"]).reshape(U * U, 1)).astype(np.float32)
    wm = p["w"] * p["sparsity_mask"]
    gw = np.zeros((U * U, 43), np.float32)
    flat = np.arange(U * U)
    jj = flat % U
    gw[flat, jj] = (wm * p["erev"]).reshape(-1)
    gw[flat, 32 + jj] = wm.reshape(-1)
    i43 = np.eye(43, dtype=np.float32)

    iS = np.arange(S)
    sigBs = np.zeros((S, S * U), np.float32)
    sigBs[iS[:, None], iS[:, None] * U + iU[None, :]] = p["sensory_sigma"]
    nms = (-(p["sensory_mu"] * p["sensory_sigma"]).reshape(S * U, 1)).astype(np.float32)
    swm = p["sensory_w"] * p["sensory_sparsity_mask"]
    gws = np.zeros((S * U, 43), np.float32)
    sflat = np.arange(S * U)
    uu = sflat % U
    gws[sflat, uu] = (swm * p["sensory_erev"]).reshape(-1)
    gws[sflat, 32 + uu] = swm.reshape(-1)

    aug = np.zeros((1, 43), np.float32)
    aug[0, :U] = p["gleak"] * p["vleak"]
    aug[0, 32:43] = p["gleak"] + EPS
    cm6 = (UNFOLDS * p["cm"]).reshape(1, U).astype(np.float32)

    mats = {
        "sigB": sigB, "negmusig": negmusig, "gw": gw, "i43": i43,
        "sigBsA": sigBs[:, :88], "sigBsB": sigBs[:, 88:],
        "nmsA": nms[:88], "nmsB": nms[88:],
        "gwsA": gws[:88], "gwsB": gws[88:],
        "aug": aug, "cm6": cm6,
    }
    cbm = np.zeros((128, CB_COLS), np.float32)
    for k, (r, o, n) in CB_LAYOUT.items():
        cbm[0:r, o:o + n] = mats[k]
    return {"cb": cbm}


def kernel(**inputs):
    from concourse.bass_utils import run_bass_kernel_spmd

    p = {k: np.asarray(v, np.float32) if np.asarray(v).dtype != np.int64
         and np.asarray(v).dtype != np.int32 else np.asarray(v)
         for k, v in inputs.items()}
    seq_lengths = np.asarray(inputs["seq_lengths"])
    inp = np.asarray(inputs["inputs"], np.float32)           # [B, T, F]

    # host-side input affine map on the S feature channels
    x = inp[:, :, :S] * p["input_w"] + p["input_b"]
    elapsed = inp[:, :, S:]
    full = np.concatenate([x, elapsed], axis=-1)             # [B, T, F]

    consts = _prep_consts(p)

    key = (T, CHUNK)
    if key not in _cache:
        _cache[key] = _build(T, CHUNK)
    nc = _cache[key]

    in_maps = []
    for cid in range(NCORES):
        sh = full[cid * BC:(cid + 1) * BC]                   # [BC, T, F]
        xsf = sh.transpose(2, 1, 0).reshape(F, T * BC)
        xs = np.zeros((33, T * BC), np.float32)
        xs[0:S] = xsf[0:S]
        xs[32] = xsf[S]
        m = {"xs": xs}
        m.update(consts)
        in_maps.append(m)

    res = run_bass_kernel_spmd(nc, in_maps, core_ids=list(range(NCORES)))

    ys = np.concatenate(
        [r["ys"].reshape(T, BC).T for r in res.results], axis=0)  # [B, T]
    seq = ys[:, :, None] * p["output_w"] + p["output_b"]          # [B, T, 1]
    idx = (seq_lengths.astype(np.int64) - 1)[:, None, None]
    last = np.take_along_axis(seq, idx, axis=1)                   # [B, 1, 1]
    out = last @ p["dense_w"] + p["dense_b"]                      # [B, 1, 1]
    return out.astype(np.float32)
